# revision 15
# baseline (speedup 1.0000x reference)
"""GQA kernel for 8x TRN2 NeuronCores (Bass/Tile), DP2 x TP4 sharding.

Layout strategy (per core; batch b = core//4, shard t = core%4):
  - x fed transposed (feature-major) xT [D, S]; projections emit token-major
    q/k/v and feature-major gate^T via PE matmuls.  x is staged in 512-token
    chunks so the gate chains run at N=512 (4 chains/chunk instead of 16
    N=128 chains -- 4x fewer PE instructions for the same cycles).
  - rmsnorm+rope token-major (free-dim reductions), then PE-transpose q,k to
    feature-major for attention. rstd is applied AFTER rope (it commutes: a
    per-(token,head) scalar) via ACT Rsqrt + one broadcast-AP DVE mul per
    tile; rope cos/sin muls are likewise single broadcast-AP ops per tile.
  - attention runs on head PAIRS (g0 head on kT/qT partitions 0-63, g1 head
    on 64-127): the two scores^T matmuls are emitted back-to-back with
    disjoint PE row-groups (tile_position (0,0) / (64,0)) into separate PSUM
    banks, so the array streams both heads concurrently (~2x scores).
  - ctx^T accumulated feature-major. Group 0 uses v_ext=[v|ones] (M=65,
    denom at psum row 64); group 1 uses v_ext=[ones|0|v] (M=128, denom at
    row 0, data at rows 64-127). Two K=1 broadcast matmuls (concurrent
    row+col tiles) expand both reciprocals into one [128,512] psum, which
    multiplies the full-width gate slab in ONE DVE op; both halves of ctxg
    are then written in-lane -- no cross-partition DMAs in normalize at all.
  - out projection token-major with ctxg stationary; y emitted bf16; partial
    [S, D] outputs summed across the 4 TP shards on host.
Local head order is interleaved (0,4,1,5,2,6,3,7) so transposed q tiles put a
g0 head on partitions 0-63 and a g1 head on 64-127, matching kT/gate/Wo
layouts without any cross-partition moves.
"""
import sys

if "/opt/trn_rl_repo" not in sys.path:
    sys.path.insert(0, "/opt/trn_rl_repo")

import numpy as np

import concourse.bass as bass
import concourse.mybir as mybir
import concourse.tile as tile
from concourse import bacc

B, S, D = 2, 2048, 2048
H, G, HD = 32, 8, 64
EPS = 1e-6
NCORES = 8
NT = S // 128          # 16 s-tiles
NQC = S // 512         # 4 q-chunks
F32 = mybir.dt.float32
BF16 = mybir.dt.bfloat16

_PERM = [0, 4, 1, 5, 2, 6, 3, 7]  # local head order (token-major col blocks)


def _bc(ap, n, where="last"):
    """stride-0 broadcast dim appended (or inserted after partition dim)."""
    if where == "last":
        return bass.AP(tensor=ap.tensor, offset=ap.offset, ap=[*ap.ap, [0, n]])
    return bass.AP(tensor=ap.tensor, offset=ap.offset,
                   ap=[ap.ap[0], [0, n], *ap.ap[1:]])


def classify_mask(mask):
    """Per (qc, kt) block class for scores^T blocks.

    Returns (classes, tiles) where classes[qc][kt] is one of
      'skip'            -- fully masked block
      ('clean',)        -- no masking
      ('tri', lo)       -- causal diagonal block: cols < lo fully masked,
                           cols [lo, lo+128) lower-triangle, rest clean
      ('mask', idx)     -- general: multiply full width by tiles[idx]
    """
    classes = []
    tiles = []
    keyidx = {}
    q_loc = np.arange(512)[:, None]
    k_loc = np.arange(128)[None, :]
    for qc in range(NQC):
        row = []
        for kt in range(NT):
            sub = mask[qc * 512:(qc + 1) * 512, kt * 128:(kt + 1) * 128]
            if sub.all():
                row.append("skip")
            elif not sub.any():
                row.append(("clean",))
            else:
                lo = kt * 128 - qc * 512
                if 0 <= lo <= 384:
                    expect = (q_loc - lo) < k_loc  # True = masked
                    if np.array_equal(sub, expect):
                        row.append(("tri", lo))
                        continue
                t = (~sub.T).astype(np.float32)  # [128k, 512q] 1=keep
                key = t.tobytes()
                if key not in keyidx:
                    keyidx[key] = len(tiles)
                    tiles.append(t)
                row.append(("mask", keyidx[key]))
        classes.append(row)
    return classes, tiles


def build_program(classes, n_masks):
    nc = bacc.Bacc("TRN2", target_bir_lowering=False, debug=False)

    def mm(out, lhsT, rhs, start, stop):
        nc.tensor.matmul(out, lhsT=lhsT, rhs=rhs, start=start, stop=stop)

    xT = nc.dram_tensor("xT", [D, S], BF16, kind="ExternalInput")
    wq = nc.dram_tensor("wq", [D, 512], BF16, kind="ExternalInput")
    wkv = nc.dram_tensor("wkv", [D, 256], BF16, kind="ExternalInput")
    wg = nc.dram_tensor("wg", [D, 512], BF16, kind="ExternalInput")
    wo = nc.dram_tensor("wo", [512, D], BF16, kind="ExternalInput")
    cosd = nc.dram_tensor("cosd", [S, HD], BF16, kind="ExternalInput")
    sind = nc.dram_tensor("sind", [S, HD], BF16, kind="ExternalInput")
    qsc = nc.dram_tensor("qsc", [128, 512], F32, kind="ExternalInput")
    ksc = nc.dram_tensor("ksc", [128, 128], F32, kind="ExternalInput")
    trit = nc.dram_tensor("trit", [128, 128], BF16, kind="ExternalInput")
    ident_in = nc.dram_tensor("ident", [128, 128], BF16, kind="ExternalInput")
    if n_masks:
        maskt = nc.dram_tensor("maskt", [n_masks, 128, 512], BF16,
                               kind="ExternalInput")
    y = nc.dram_tensor("y", [S, D], BF16, kind="ExternalOutput")

    AF = mybir.ActivationFunctionType
    from contextlib import ExitStack
    with tile.TileContext(nc) as tc, ExitStack() as es:
        singles = es.enter_context(tc.tile_pool(name="singles", bufs=1))
        xpool = es.enter_context(tc.tile_pool(name="xpool", bufs=2))
        pwork = es.enter_context(tc.tile_pool(name="pwork", bufs=2))
        psum = es.enter_context(tc.tile_pool(name="psum", bufs=1, space="PSUM"))
        awork = es.enter_context(tc.tile_pool(name="awork", bufs=3, space="SBUF"))

        # ---- resident constants / weights; order = startup DMA priority ----
        # wq/wkv/x arrive in interleaved plane-groups so the first (q,kv)
        # chain can start after ~1/4 of the weight bytes instead of all
        wq_sb = singles.tile([128, NT, 512], BF16)
        wq_r = wq.ap().rearrange("(a p) n -> p a n", p=128)
        wkv_sb = singles.tile([128, NT, 256], BF16)
        wkv_r = wkv.ap().rearrange("(a p) n -> p a n", p=128)

        xch = {}

        def load_xchunk(c, half=None):
            if c in xch:
                t = xch[c]
            else:
                t = xpool.tile([128, NT, 512], BF16, tag="xch", name=f"xch_{c}")
                xch[c] = t
            halves = (0, 1) if half is None else (half,)
            for k in halves:
                nc.sync.dma_start(
                    out=t[:, :, k * 256:(k + 1) * 256],
                    in_=xT.ap()[:, c * 512 + k * 256:c * 512 + (k + 1) * 256]
                    .rearrange("(a p) m -> p a m", p=128))

        nc.sync.dma_start(out=wq_sb[:, 0:8, :], in_=wq_r[:, 0:8, :])
        nc.sync.dma_start(out=wkv_sb[:, 0:8, :], in_=wkv_r[:, 0:8, :])
        load_xchunk(0, half=0)
        nc.sync.dma_start(out=wq_sb[:, 8:16, :], in_=wq_r[:, 8:16, :])
        nc.sync.dma_start(out=wkv_sb[:, 8:16, :], in_=wkv_r[:, 8:16, :])
        qsc_sb = singles.tile([128, 512], F32)
        nc.sync.dma_start(out=qsc_sb, in_=qsc.ap())
        ksc_sb = singles.tile([128, 128], F32)
        nc.sync.dma_start(out=ksc_sb, in_=ksc.ap())
        load_xchunk(0, half=1)
        cos_sb = singles.tile([128, NT, HD], BF16)
        nc.sync.dma_start(out=cos_sb, in_=cosd.ap().rearrange("(a p) n -> p a n", p=128))
        sin_sb = singles.tile([128, NT, HD], BF16)
        nc.sync.dma_start(out=sin_sb, in_=sind.ap().rearrange("(a p) n -> p a n", p=128))
        ident_sb = singles.tile([128, 128], BF16)
        nc.sync.dma_start(out=ident_sb, in_=ident_in.ap())
        tri_sb = singles.tile([128, 128], BF16)
        nc.sync.dma_start(out=tri_sb, in_=trit.ap())
        wg_sb = singles.tile([128, NT, 512], BF16)
        nc.sync.dma_start(out=wg_sb, in_=wg.ap().rearrange("(a p) n -> p a n", p=128))
        if n_masks:
            mask_sb = singles.tile([128, n_masks, 512], BF16)
            nc.sync.dma_start(out=mask_sb,
                              in_=maskt.ap().rearrange("a p n -> p a n"))
        # wo load deferred (emitted before the first out-projection) so it
        # does not block the x-chunk streaming DMAs at startup
        wo_sb = singles.tile([128, 4, D], BF16)
        wo_loaded = []

        def load_wo():
            if not wo_loaded:
                nc.sync.dma_start(
                    out=wo_sb, in_=wo.ap().rearrange("(a p) n -> p a n", p=128))
                wo_loaded.append(True)

        qT = singles.tile([128, 4, S], BF16)       # head nt @0-63, 4+nt @64-127
        kT = singles.tile([128, S], BF16)          # group0 @0-63, group1 @64-127
        vexta = singles.tile([128, NT, 65], BF16)   # [v(64) | ones]
        nc.vector.memset(vexta[:, :, 64], 1.0)
        vextb = singles.tile([128, NT, 128], BF16)  # [ones | 0*63 | v(64)]
        nc.vector.memset(vextb[:, :, 0:64], 0.0)
        nc.vector.memset(vextb[:, :, 0], 1.0)
        eps_sb = singles.tile([128, 1], F32)
        nc.vector.memset(eps_sb, float(EPS))
        ones_sb = singles.tile([128, 64], BF16)
        nc.vector.memset(ones_sb, 1.0)
        graw = singles.tile([128, 4, S], BF16)     # gate^T; sigmoid in-place

        def emit_g_chunk(qc):
            xc = xch[qc]
            for nt in range(4):
                g_ps = psum.tile([128, 512], F32, tag="ps_to", bufs=2,
                                 name=f"gps_{qc}_{nt}")
                for dt_ in range(NT):
                    mm(g_ps, wg_sb[:, dt_, nt * 128:(nt + 1) * 128],
                       rhs=xc[:, dt_, :],
                       start=(dt_ == 0), stop=(dt_ == NT - 1))
                nc.scalar.copy(graw[:, nt, qc * 512:(qc + 1) * 512], g_ps)

        def phase_p_tile(i):
            """projections + rmsnorm + rope (rstd deferred) for s-tile i."""
            c, quarter = i // 4, i % 4
            if quarter == 0 and c + 1 < NQC and (c + 1) not in xch:
                load_xchunk(c + 1)
            xt = xch[c][:, :, quarter * 128:(quarter + 1) * 128]

            # q and kv interleaved per dt_: the stationary operand (the x
            # tile) is shared, so the PE reloads weights once per dt_ instead
            # of twice and the ldweights stream stays off the critical path
            q_ps = psum.tile([128, 512], F32, tag="ps_qp", bufs=1, name=f"qps_{i}")
            kv_ps = psum.tile([128, 256], F32, tag="ps_to", bufs=2, name=f"kvps_{i}")
            for dt_ in range(NT):
                mm(q_ps, xt[:, dt_, :], rhs=wq_sb[:, dt_, :],
                   start=(dt_ == 0), stop=(dt_ == NT - 1))
                mm(kv_ps, xt[:, dt_, :], rhs=wkv_sb[:, dt_, :],
                   start=(dt_ == 0), stop=(dt_ == NT - 1))

            # ---- q: square+reduce (rstd later), scale, rope ----
            q3 = q_ps.rearrange("p (h e) -> p h e", e=64)
            sq = pwork.tile([128, 8, 64], BF16, tag="sq")
            nc.scalar.square(sq, q3)
            nc.vector.reduce_sum(ssq[:, i % 4, :], sq, axis=mybir.AxisListType.X)
            qn = pwork.tile([128, 8, 64], BF16, tag="qn")
            nc.vector.tensor_mul(qn.rearrange("p h e -> p (h e)"), q_ps, qsc_sb)
            rot = pwork.tile([128, 8, 64], BF16, tag="rot")
            nc.vector.tensor_scalar_mul(rot[:, :, 0:32], qn[:, :, 32:64], -1.0)
            nc.vector.tensor_copy(rot[:, :, 32:64], qn[:, :, 0:32])
            qf = qfc[i % 4]
            nc.vector.tensor_mul(qf, qn, _bc(cos_sb[:, i, :], 8, "mid"))
            nc.vector.tensor_mul(rot, rot, _bc(sin_sb[:, i, :], 8, "mid"))
            nc.vector.tensor_add(qf.rearrange("p h e -> p (h e)"),
                                 qf.rearrange("p h e -> p (h e)"),
                                 rot.rearrange("p h e -> p (h e)"))

            # ---- k ----
            k3 = kv_ps[:, 0:128].rearrange("p (h e) -> p h e", e=64)
            ksq = pwork.tile([128, 2, 64], BF16, tag="ksq")
            nc.scalar.square(ksq, k3)
            nc.vector.reduce_sum(ssk[:, i % 4, :], ksq, axis=mybir.AxisListType.X)
            kn = pwork.tile([128, 2, 64], BF16, tag="kn")
            nc.vector.tensor_mul(kn.rearrange("p h e -> p (h e)"),
                                 kv_ps[:, 0:128], ksc_sb)
            krot = pwork.tile([128, 2, 64], BF16, tag="krot")
            nc.vector.tensor_scalar_mul(krot[:, :, 0:32], kn[:, :, 32:64], -1.0)
            nc.vector.tensor_copy(krot[:, :, 32:64], kn[:, :, 0:32])
            kf = kfc[i % 4]
            nc.vector.tensor_mul(kf, kn, _bc(cos_sb[:, i, :], 2, "mid"))
            nc.vector.tensor_mul(krot, krot, _bc(sin_sb[:, i, :], 2, "mid"))
            nc.vector.tensor_add(kf.rearrange("p h e -> p (h e)"),
                                 kf.rearrange("p h e -> p (h e)"),
                                 krot.rearrange("p h e -> p (h e)"))

            # v into v_ext tiles (cast to bf16)
            nc.scalar.copy(vexta[:, i, 0:64], kv_ps[:, 128:192])
            nc.scalar.copy(vextb[:, i, 64:128], kv_ps[:, 192:256])

        def tail_gate(qc):
            """sigmoid on the chunk's gate slab (depends on the gate chains)"""
            gsl = graw[:, :, qc * 512:(qc + 1) * 512]
            nc.scalar.activation(gsl, gsl, AF.Sigmoid)

        def phase_p_chunk_tail(qc):
            """batched rsqrt + rstd application + transposes.

            Depends only on the chunk's four P tiles (not on the gate
            chains), so it is emitted as early as possible: the next chunk's
            attention pairs wait on the qT/kT transposes emitted here."""
            nc.scalar.activation(ssq, ssq, AF.Sqrt, bias=eps_sb, scale=1.0 / 64)
            nc.scalar.activation(ssk, ssk, AF.Sqrt, bias=eps_sb, scale=1.0 / 64)
            nc.vector.reciprocal(ssq, ssq)
            nc.vector.reciprocal(ssk, ssk)
            for u in range(4):
                i = qc * 4 + u
                qf, kf = qfc[u], kfc[u]
                nc.vector.tensor_mul(qf, qf, _bc(ssq[:, u, :], 64))
                nc.vector.tensor_mul(kf, kf, _bc(ssk[:, u, :], 64))
                qf2 = qf.rearrange("p h e -> p (h e)")
                for nt in range(4):
                    tp = psum.tile([128, 128], BF16, tag="ps_s", bufs=3,
                                   name=f"tp_{i}_{nt}")
                    nc.tensor.transpose(tp, qf2[:, nt * 128:(nt + 1) * 128],
                                        ident_sb)
                    nc.vector.tensor_copy(qT[:, nt, i * 128:(i + 1) * 128], tp)
                kf2 = kf.rearrange("p h e -> p (h e)")
                tpk = psum.tile([128, 128], BF16, tag="ps_s", bufs=3,
                                name=f"tpk_{i}")
                nc.tensor.transpose(tpk, kf2, ident_sb)
                nc.vector.tensor_copy(kT[:, i * 128:(i + 1) * 128], tpk)

        def a_pair(qc, nt, ctxg):
            """attention for head pair (g0 head nt, g1 head 4+nt) of chunk qc.

            Scores for the two heads are emitted back-to-back as disjoint
            PE row-group tiles (kT partitions 0-63 vs 64-127) into separate
            PSUM banks, so they stream through the array concurrently."""
            kts = [kt for kt in range(NT) if classes[qc][kt] != "skip"]
            ctxA = psum.tile([128, 512], F32, tag="ps_ctx", bufs=2,
                             name=f"ctxA_{qc}_{nt}")
            ctxB = psum.tile([128, 512], F32, tag="ps_ctx", bufs=2,
                             name=f"ctxB_{qc}_{nt}")

            def emit_score(j):
                kt = kts[j]
                cls = classes[qc][kt]
                lo = cls[1] if cls[0] == "tri" else 0
                ktw = kT[:, kt * 128:(kt + 1) * 128]
                qw = qT[:, nt, qc * 512 + lo:(qc + 1) * 512]
                sA = psum.tile([128, 512], F32, tag="ps_s", bufs=3,
                               name=f"sA_{qc}_{nt}_{kt}")
                sB = psum.tile([128, 512], F32, tag="ps_s", bufs=3,
                               name=f"sB_{qc}_{nt}_{kt}")
                mm(sA[:, lo:512], ktw[0:64, :], rhs=qw[0:64, :],
                   start=True, stop=True)
                mm(sB[:, lo:512], ktw[64:128, :], rhs=qw[64:128, :],
                   start=True, stop=True)
                # one sbuf tile for both heads: the tri/mask multiply then
                # covers the pair in a single strided DVE op
                eAB = awork.tile([128, 2, 512], BF16, tag="eT", bufs=3)
                nc.scalar.activation(eAB[:, 0, lo:512], sA[:, lo:512], AF.Exp)
                nc.scalar.activation(eAB[:, 1, lo:512], sB[:, lo:512], AF.Exp)
                if cls[0] == "tri":
                    nc.vector.tensor_mul(eAB[:, :, lo:lo + 128],
                                         eAB[:, :, lo:lo + 128],
                                         _bc(tri_sb[:, :], 2, "mid"))
                elif cls[0] == "mask":
                    nc.vector.tensor_mul(eAB, eAB,
                                         _bc(mask_sb[:, cls[1], :], 2, "mid"))
                return eAB, lo

            def emit_ctx(j, eAB, lo):
                last = (j == len(kts) - 1)
                mm(ctxA[0:65, lo:512], vexta[:, kts[j], :],
                   rhs=eAB[:, 0, lo:512], start=(j == 0), stop=last)
                mm(ctxB[:, lo:512], vextb[:, kts[j], :],
                   rhs=eAB[:, 1, lo:512], start=(j == 0), stop=last)

            # scores emitted one step ahead of the ctx accumulation so the
            # PE stream never head-of-line blocks on an exp in flight
            pend = None
            for j in range(len(kts)):
                cur = emit_score(j)
                if pend is not None:
                    emit_ctx(j - 1, *pend)
                pend = cur
            emit_ctx(len(kts) - 1, *pend)

            def normalize():
                # denomA at ctxA row 64; denomB at ctxB row 0 (vextb ones@0).
                # Two K=1 broadcast matmuls (disjoint row+col groups, same
                # bank) expand both reciprocals to [128,512]; one DVE mul
                # applies the full-width gate slab; both ctxg halves write
                # in-lane (ctxB data lives at rows 64-127).
                rstage = awork.tile([128, 512], BF16, tag="rstage", bufs=2)
                with nc.allow_low_precision(reason="bf16 softmax denom"):
                    nc.vector.reciprocal(rstage[64:65, :], ctxA[64:65, :])
                    nc.vector.reciprocal(rstage[0:1, :], ctxB[0:1, :])
                rb_ps = psum.tile([128, 512], F32, tag="ps_s", bufs=3,
                                  name=f"rbps_{qc}_{nt}")
                mm(rb_ps[0:64, :], ones_sb[64:65, :],
                   rhs=rstage[64:65, :], start=True, stop=True)
                mm(rb_ps[64:128, :], ones_sb[0:1, :],
                   rhs=rstage[0:1, :], start=True, stop=True)
                m1 = awork.tile([128, 512], BF16, tag="m1", bufs=2)
                nc.vector.tensor_mul(m1, rb_ps,
                                     graw[:, nt, qc * 512:(qc + 1) * 512])
                nc.vector.tensor_mul(ctxg[nt][0:64, :], ctxA[0:64, :],
                                     m1[0:64, :])
                nc.vector.tensor_mul(ctxg[nt][64:128, :], ctxB[64:128, :],
                                     m1[64:128, :])
            return normalize

        def a_outproj_ssub(qc, ctxg, ssub):
            """output projection for one 128-row slab of q-chunk qc."""
            srow = qc * 512 + ssub * 128
            ostage = awork.tile([128, D], BF16, tag="ostage", bufs=3)
            for dc in range(4):
                o_ps = psum.tile([128, 512], F32, tag="ps_to", bufs=2,
                                 name=f"ops_{qc}_{ssub}_{dc}")
                for nt in range(4):
                    mm(o_ps, ctxg[nt][:, ssub * 128:(ssub + 1) * 128],
                       rhs=wo_sb[:, nt, dc * 512:(dc + 1) * 512],
                       start=(nt == 0), stop=(nt == 3))
                # split the psum drains between ACT and DVE (copy lives in
                # every ACT table set, so no table reloads are triggered)
                if dc % 2 == (0 if qc == NQC - 1 else 1):
                    nc.scalar.copy(ostage[:, dc * 512:(dc + 1) * 512], o_ps)
                else:
                    nc.vector.tensor_copy(ostage[:, dc * 512:(dc + 1) * 512],
                                          o_ps)
            nc.sync.dma_start(out=y.ap()[srow:srow + 128, :], in_=ostage)

        # ======== main schedule: P chunk -> (tail) -> A chunk, interleaved ==
        ssq = singles.tile([128, 4, 8], F32)
        ssk = singles.tile([128, 4, 2], F32)
        qfc = [singles.tile([128, 8, 64], BF16, name=f"qfc{u}") for u in range(4)]
        kfc = [singles.tile([128, 2, 64], BF16, name=f"kfc{u}") for u in range(4)]

        # PE warm-up on memset data while the first weight/x DMAs stream in:
        # ~4us of array activity flips the HAM clock gate to 8/8 (2.4 GHz)
        # before the first real projection chain issues, and costs nothing
        # (the PE would be idle waiting on DMA anyway).
        warm_rhs = vextb[:, 0:8, 0:64]
        for w in range(12):
            warm_ps = psum.tile([64, 512], F32, tag="ps_s", bufs=3,
                                name=f"warm_{w}")
            mm(warm_ps, ones_sb, rhs=warm_rhs, start=True, stop=True)

        # A(qc) head-pairs interleaved with P tiles of chunk qc+1 and the
        # out-projection slabs of chunk qc-1, so the PE always has dense
        # projection matmuls to chew on while ACT runs the exps.  The gate
        # chunk + P tail of chunk qc+1 and two trailing out-projection slabs
        # land after the pairs: PE-dense work that fills the tail's
        # sigmoid/rsqrt/transpose window.
        for u in range(4):
            phase_p_tile(u)
        emit_g_chunk(0)
        phase_p_chunk_tail(0)
        tail_gate(0)
        prev = None  # (qc, ctxg) awaiting out-projection
        for qc in range(NQC):
            load_wo()
            ctxg = [awork.tile([128, 512], BF16, tag=f"ctxg{nt}",
                               name=f"ctxg{nt}_{qc}", bufs=2)
                    for nt in range(4)]
            for u in range(4):
                norm = a_pair(qc, u, ctxg)
                norm()
                # P tiles of chunk qc+1 front-loaded into the first two
                # pairs so the transpose tail can start two pairs early;
                # the later pairs interleave with gate chains + out-proj
                if qc + 1 < NQC:
                    if u < 2:
                        phase_p_tile((qc + 1) * 4 + 2 * u)
                        phase_p_tile((qc + 1) * 4 + 2 * u + 1)
                    elif u == 2:
                        phase_p_chunk_tail(qc + 1)
                    else:
                        emit_g_chunk(qc + 1)
                if prev is not None and u == 2:
                    a_outproj_ssub(prev[0], prev[1], 0)
                if prev is not None and u == 3:
                    a_outproj_ssub(prev[0], prev[1], 1)
            if qc + 1 < NQC:
                tail_gate(qc + 1)
            if prev is not None:
                a_outproj_ssub(prev[0], prev[1], 2)
                a_outproj_ssub(prev[0], prev[1], 3)
            prev = (qc, ctxg)
        for ssub in range(4):
            a_outproj_ssub(prev[0], prev[1], ssub)

    nc.compile()
    return nc


# ======================== host-side runner =================================
_CACHE = {}


class _Runner:
    """Jitted sharded executable for a prebuilt Bass module, reusable."""

    def __init__(self, nc, n_cores):
        import jax
        import numpy as _np
        from jax.sharding import Mesh, PartitionSpec
        from jax.experimental.shard_map import shard_map
        from concourse.bass2jax import (_bass_exec_p, partition_id_tensor,
                                        install_neuronx_cc_hook)
        install_neuronx_cc_hook()
        self.jax = jax
        self.nc = nc
        self.n_cores = n_cores
        partition_name = (nc.partition_id_tensor.name
                          if nc.partition_id_tensor else None)
        in_names, out_names, out_avals = [], [], []
        for alloc in nc.m.functions[0].allocations:
            if not isinstance(alloc, mybir.MemoryLocationSet):
                continue
            name = alloc.memorylocations[0].name
            if alloc.kind == "ExternalInput":
                if name != partition_name:
                    in_names.append(name)
            elif alloc.kind == "ExternalOutput":
                out_names.append(name)
                out_avals.append(jax.core.ShapedArray(
                    tuple(alloc.tensor_shape), mybir.dt.np(alloc.dtype)))
        self.in_names, self.out_names, self.out_avals = in_names, out_names, out_avals
        all_in = list(in_names) + list(out_names)
        if partition_name is not None:
            all_in.append(partition_name)
        self._partition_name = partition_name
        self._all_in = all_in
        self._dbg_name = nc.dbg_addr.name if nc.dbg_addr is not None else None

        devices = jax.devices()[:n_cores]
        self.mesh = Mesh(_np.asarray(devices), ("core",))
        self.fn = None
        self.dev_in = None

    def _compile_fast(self):
        """AOT-compile the shard_map body on the effect-free C++ fast-dispatch
        path (saves ~700us/exec of python dispatch overhead)."""
        import jax
        from jax.sharding import PartitionSpec
        from jax.experimental.shard_map import shard_map
        import concourse.bass2jax as b2j
        nc = self.nc
        partition_name = self._partition_name
        all_in, out_names, out_avals = self._all_in, self.out_names, self.out_avals

        def _body(*args):
            operands = list(args)
            if partition_name is not None:
                operands.append(b2j.partition_id_tensor())
            outs = b2j._bass_exec_p.bind(
                *operands, out_avals=tuple(out_avals), in_names=tuple(all_in),
                out_names=tuple(out_names), lowering_input_output_aliases=(),
                sim_require_finite=True, sim_require_nnan=True, nc=nc)
            return tuple(outs)

        n = len(self.in_names) + len(out_names)

        def compile_fn():
            f = jax.jit(shard_map(
                _body, mesh=self.mesh,
                in_specs=(PartitionSpec("core"),) * n,
                out_specs=(PartitionSpec("core"),) * len(out_names),
                check_rep=False))
            return f.lower(*self.dev_in).compile()

        try:
            self.fn = b2j.fast_dispatch_compile(compile_fn)
        except Exception:
            f = jax.jit(shard_map(
                _body, mesh=self.mesh,
                in_specs=(PartitionSpec("core"),) * n,
                out_specs=(PartitionSpec("core"),) * len(out_names),
                check_rep=False))
            self.fn = f

    def prepare(self, in_maps):
        import numpy as _np
        from jax.sharding import NamedSharding, PartitionSpec
        if self._dbg_name is not None:
            in_maps = [{**m, self._dbg_name: _np.zeros((1, 2), _np.uint32)}
                       for m in in_maps]
        concat = [_np.concatenate([_np.asarray(in_maps[c][n])
                                   for c in range(self.n_cores)], axis=0)
                  for n in self.in_names]
        # zero output buffers: device-resident, NOT donated, reused each run.
        # Valid because the kernel writes every element of its outputs.
        concat += [_np.zeros((self.n_cores * av.shape[0], *av.shape[1:]),
                             av.dtype) for av in self.out_avals]
        shard = NamedSharding(self.mesh, PartitionSpec("core"))
        self.dev_in = [self.jax.device_put(a, shard) for a in concat]
        if self.fn is None:
            self._compile_fast()
        return self

    def run(self):
        return self.jax.block_until_ready(self.fn(*self.dev_in))

    def results(self, outs):
        import numpy as _np
        res = []
        for c in range(self.n_cores):
            d = {}
            for i, name in enumerate(self.out_names):
                full = _np.asarray(outs[i])
                d[name] = full.reshape(self.n_cores, *self.out_avals[i].shape)[c]
            res.append(d)
        return res


def make_runner(nc, n_cores):
    return _Runner(nc, n_cores)


def _prep_core_inputs(inputs, b, t, shared):
    x = inputs["x"]
    import ml_dtypes
    bf = ml_dtypes.bfloat16

    if ("xT", b) not in shared:
        shared[("xT", b)] = np.ascontiguousarray(np.asarray(x[b]).T).astype(bf)
    if ("w", t) not in shared:
        Wq, Wk, Wv, Wg, Wo = (np.asarray(inputs[k])
                              for k in ("Wq", "Wk", "Wv", "Wg", "Wo"))
        heads = [8 * t + p for p in _PERM]
        qcols = np.concatenate([np.arange(h * 64, (h + 1) * 64) for h in heads])
        groups = [2 * t, 2 * t + 1]
        kcols = np.concatenate([np.arange(g * 64, (g + 1) * 64) for g in groups])
        shared[("w", t)] = {
            "wq": np.ascontiguousarray(Wq[:, qcols]).astype(bf),
            "wkv": np.ascontiguousarray(
                np.concatenate([Wk[:, kcols], Wv[:, kcols]], axis=1)).astype(bf),
            "wg": np.ascontiguousarray(Wg[:, qcols]).astype(bf),
            "wo": np.ascontiguousarray(Wo[qcols, :]).astype(bf),
        }
    if "const" not in shared:
        q_scale, k_scale = np.asarray(inputs["q_scale"]), np.asarray(inputs["k_scale"])
        cos, sin = np.asarray(inputs["cos"]), np.asarray(inputs["sin"])
        scaling = float(HD) ** -0.5
        tri = (np.arange(128)[:, None] <= np.arange(128)[None, :])
        shared["const"] = {
            "cosd": cos.astype(bf), "sind": sin.astype(bf),
            "qsc": np.broadcast_to(
                np.tile((1.0 + q_scale) * scaling, 8)[None, :],
                (128, 512)).astype(np.float32).copy(),
            "ksc": np.broadcast_to(
                np.tile(1.0 + k_scale, 2)[None, :],
                (128, 128)).astype(np.float32).copy(),
            "trit": tri.astype(bf),
            "ident": np.eye(128, dtype=np.float32).astype(bf),
        }
    return {"xT": shared[("xT", b)], **shared[("w", t)], **shared["const"]}


def kernel(**inputs):
    mask = np.asarray(inputs["mask"])
    classes, tiles = classify_mask(mask)
    key = mask.tobytes()
    if key not in _CACHE:
        nc = build_program(classes, len(tiles))
        _CACHE[key] = (nc, make_runner(nc, NCORES))
    nc, runner = _CACHE[key]

    import ml_dtypes
    mask_arr = (np.stack(tiles).astype(ml_dtypes.bfloat16) if tiles else None)
    shared = {}
    in_maps = []
    for c in range(NCORES):
        m = _prep_core_inputs(inputs, c // 4, c % 4, shared)
        if mask_arr is not None:
            m["maskt"] = mask_arr
        in_maps.append(m)

    runner.prepare(in_maps)
    outs = runner.run()
    res = runner.results(outs)
    out = np.zeros((B, S, D), np.float32)
    for c in range(NCORES):
        out[c // 4] += res[c]["y"].astype(np.float32)
    return out.astype(np.asarray(inputs["x"]).dtype)


# revision 16
# speedup vs baseline: 1.0097x; 1.0097x over previous
"""GQA kernel for 8x TRN2 NeuronCores (Bass/Tile), DP2 x TP4 sharding.

Layout strategy (per core; batch b = core//4, shard t = core%4):
  - x fed transposed (feature-major) xT [D, S]; projections emit token-major
    q/k/v and feature-major gate^T via PE matmuls.  x is staged in 512-token
    chunks so the gate chains run at N=512 (4 chains/chunk instead of 16
    N=128 chains -- 4x fewer PE instructions for the same cycles).
  - rmsnorm+rope token-major (free-dim reductions), then PE-transpose q,k to
    feature-major for attention. rstd is applied AFTER rope (it commutes: a
    per-(token,head) scalar) via ACT Rsqrt + one broadcast-AP DVE mul per
    tile; rope cos/sin muls are likewise single broadcast-AP ops per tile.
  - attention runs on head PAIRS (g0 head on kT/qT partitions 0-63, g1 head
    on 64-127): the two scores^T matmuls are emitted back-to-back with
    disjoint PE row-groups (tile_position (0,0) / (64,0)) into separate PSUM
    banks, so the array streams both heads concurrently (~2x scores).
  - ctx^T accumulated feature-major. Group 0 uses v_ext=[v|ones] (M=65,
    denom at psum row 64); group 1 uses v_ext=[ones|0|v] (M=128, denom at
    row 0, data at rows 64-127). Two K=1 broadcast matmuls (concurrent
    row+col tiles) expand both reciprocals into one [128,512] psum, which
    multiplies the full-width gate slab in ONE DVE op; both halves of ctxg
    are then written in-lane -- no cross-partition DMAs in normalize at all.
  - out projection token-major with ctxg stationary; y emitted bf16; partial
    [S, D] outputs summed across the 4 TP shards on host.
Local head order is interleaved (0,4,1,5,2,6,3,7) so transposed q tiles put a
g0 head on partitions 0-63 and a g1 head on 64-127, matching kT/gate/Wo
layouts without any cross-partition moves.
"""
import sys

if "/opt/trn_rl_repo" not in sys.path:
    sys.path.insert(0, "/opt/trn_rl_repo")

import numpy as np

import concourse.bass as bass
import concourse.mybir as mybir
import concourse.tile as tile
from concourse import bacc

B, S, D = 2, 2048, 2048
H, G, HD = 32, 8, 64
EPS = 1e-6
NCORES = 8
NT = S // 128          # 16 s-tiles
NQC = S // 512         # 4 q-chunks
F32 = mybir.dt.float32
BF16 = mybir.dt.bfloat16

_PERM = [0, 4, 1, 5, 2, 6, 3, 7]  # local head order (token-major col blocks)


def _bc(ap, n, where="last"):
    """stride-0 broadcast dim appended (or inserted after partition dim)."""
    if where == "last":
        return bass.AP(tensor=ap.tensor, offset=ap.offset, ap=[*ap.ap, [0, n]])
    return bass.AP(tensor=ap.tensor, offset=ap.offset,
                   ap=[ap.ap[0], [0, n], *ap.ap[1:]])


def classify_mask(mask):
    """Per (qc, kt) block class for scores^T blocks.

    Returns (classes, tiles) where classes[qc][kt] is one of
      'skip'            -- fully masked block
      ('clean',)        -- no masking
      ('tri', lo)       -- causal diagonal block: cols < lo fully masked,
                           cols [lo, lo+128) lower-triangle, rest clean
      ('mask', idx)     -- general: multiply full width by tiles[idx]
    """
    classes = []
    tiles = []
    keyidx = {}
    q_loc = np.arange(512)[:, None]
    k_loc = np.arange(128)[None, :]
    for qc in range(NQC):
        row = []
        for kt in range(NT):
            sub = mask[qc * 512:(qc + 1) * 512, kt * 128:(kt + 1) * 128]
            if sub.all():
                row.append("skip")
            elif not sub.any():
                row.append(("clean",))
            else:
                lo = kt * 128 - qc * 512
                if 0 <= lo <= 384:
                    expect = (q_loc - lo) < k_loc  # True = masked
                    if np.array_equal(sub, expect):
                        row.append(("tri", lo))
                        continue
                t = (~sub.T).astype(np.float32)  # [128k, 512q] 1=keep
                key = t.tobytes()
                if key not in keyidx:
                    keyidx[key] = len(tiles)
                    tiles.append(t)
                row.append(("mask", keyidx[key]))
        classes.append(row)
    return classes, tiles


def build_program(classes, n_masks):
    nc = bacc.Bacc("TRN2", target_bir_lowering=False, debug=False)

    def mm(out, lhsT, rhs, start, stop):
        nc.tensor.matmul(out, lhsT=lhsT, rhs=rhs, start=start, stop=stop)

    xT = nc.dram_tensor("xT", [D, S], BF16, kind="ExternalInput")
    wq = nc.dram_tensor("wq", [D, 512], BF16, kind="ExternalInput")
    wkv = nc.dram_tensor("wkv", [D, 256], BF16, kind="ExternalInput")
    wg = nc.dram_tensor("wg", [D, 512], BF16, kind="ExternalInput")
    wo = nc.dram_tensor("wo", [512, D], BF16, kind="ExternalInput")
    cosd = nc.dram_tensor("cosd", [S, HD], BF16, kind="ExternalInput")
    sind = nc.dram_tensor("sind", [S, HD], BF16, kind="ExternalInput")
    qsc = nc.dram_tensor("qsc", [128, 512], F32, kind="ExternalInput")
    ksc = nc.dram_tensor("ksc", [128, 128], F32, kind="ExternalInput")
    trit = nc.dram_tensor("trit", [128, 128], BF16, kind="ExternalInput")
    ident_in = nc.dram_tensor("ident", [128, 128], BF16, kind="ExternalInput")
    if n_masks:
        maskt = nc.dram_tensor("maskt", [n_masks, 128, 512], BF16,
                               kind="ExternalInput")
    y = nc.dram_tensor("y", [S, D], BF16, kind="ExternalOutput")

    AF = mybir.ActivationFunctionType
    from contextlib import ExitStack
    with tile.TileContext(nc) as tc, ExitStack() as es:
        singles = es.enter_context(tc.tile_pool(name="singles", bufs=1))
        xpool = es.enter_context(tc.tile_pool(name="xpool", bufs=2))
        pwork = es.enter_context(tc.tile_pool(name="pwork", bufs=2))
        psum = es.enter_context(tc.tile_pool(name="psum", bufs=1, space="PSUM"))
        awork = es.enter_context(tc.tile_pool(name="awork", bufs=3, space="SBUF"))

        # ---- resident constants / weights; order = startup DMA priority ----
        # wq/wkv/x arrive in interleaved plane-groups so the first (q,kv)
        # chain can start after ~1/4 of the weight bytes instead of all
        wq_sb = singles.tile([128, NT, 512], BF16)
        wq_r = wq.ap().rearrange("(a p) n -> p a n", p=128)
        wkv_sb = singles.tile([128, NT, 256], BF16)
        wkv_r = wkv.ap().rearrange("(a p) n -> p a n", p=128)

        xch = {}

        def load_xchunk(c, half=None):
            if c in xch:
                t = xch[c]
            else:
                t = xpool.tile([128, NT, 512], BF16, tag="xch", name=f"xch_{c}")
                xch[c] = t
            halves = (0, 1) if half is None else (half,)
            for k in halves:
                nc.sync.dma_start(
                    out=t[:, :, k * 256:(k + 1) * 256],
                    in_=xT.ap()[:, c * 512 + k * 256:c * 512 + (k + 1) * 256]
                    .rearrange("(a p) m -> p a m", p=128))

        nc.sync.dma_start(out=wq_sb[:, 0:8, :], in_=wq_r[:, 0:8, :])
        nc.sync.dma_start(out=wkv_sb[:, 0:8, :], in_=wkv_r[:, 0:8, :])
        load_xchunk(0, half=0)
        nc.sync.dma_start(out=wq_sb[:, 8:16, :], in_=wq_r[:, 8:16, :])
        nc.sync.dma_start(out=wkv_sb[:, 8:16, :], in_=wkv_r[:, 8:16, :])
        qsc_sb = singles.tile([128, 512], F32)
        nc.sync.dma_start(out=qsc_sb, in_=qsc.ap())
        ksc_sb = singles.tile([128, 128], F32)
        nc.sync.dma_start(out=ksc_sb, in_=ksc.ap())
        load_xchunk(0, half=1)
        cos_sb = singles.tile([128, NT, HD], BF16)
        nc.sync.dma_start(out=cos_sb, in_=cosd.ap().rearrange("(a p) n -> p a n", p=128))
        sin_sb = singles.tile([128, NT, HD], BF16)
        nc.sync.dma_start(out=sin_sb, in_=sind.ap().rearrange("(a p) n -> p a n", p=128))
        ident_sb = singles.tile([128, 128], BF16)
        nc.sync.dma_start(out=ident_sb, in_=ident_in.ap())
        tri_sb = singles.tile([128, 128], BF16)
        nc.sync.dma_start(out=tri_sb, in_=trit.ap())
        wg_sb = singles.tile([128, NT, 512], BF16)
        nc.sync.dma_start(out=wg_sb, in_=wg.ap().rearrange("(a p) n -> p a n", p=128))
        if n_masks:
            mask_sb = singles.tile([128, n_masks, 512], BF16)
            nc.sync.dma_start(out=mask_sb,
                              in_=maskt.ap().rearrange("a p n -> p a n"))
        # wo load deferred (emitted before the first out-projection) so it
        # does not block the x-chunk streaming DMAs at startup
        wo_sb = singles.tile([128, 4, D], BF16)
        wo_loaded = []

        def load_wo():
            if not wo_loaded:
                nc.sync.dma_start(
                    out=wo_sb, in_=wo.ap().rearrange("(a p) n -> p a n", p=128))
                wo_loaded.append(True)

        qT = singles.tile([128, 4, S], BF16)       # head nt @0-63, 4+nt @64-127
        kT = singles.tile([128, S], BF16)          # group0 @0-63, group1 @64-127
        vexta = singles.tile([128, NT, 65], BF16)   # [v(64) | ones]
        nc.vector.memset(vexta[:, :, 64], 1.0)
        vextb = singles.tile([128, NT, 128], BF16)  # [ones | 0*63 | v(64)]
        nc.vector.memset(vextb[:, :, 0:64], 0.0)
        nc.vector.memset(vextb[:, :, 0], 1.0)
        eps_sb = singles.tile([128, 1], F32)
        nc.vector.memset(eps_sb, float(EPS))
        ones_sb = singles.tile([128, 64], BF16)
        nc.vector.memset(ones_sb, 1.0)
        graw = singles.tile([128, 4, S], BF16)     # gate^T; sigmoid in-place

        def emit_g_chunk(qc):
            xc = xch[qc]
            for nt in range(4):
                g_ps = psum.tile([128, 512], F32, tag="ps_to", bufs=2,
                                 name=f"gps_{qc}_{nt}")
                for dt_ in range(NT):
                    mm(g_ps, wg_sb[:, dt_, nt * 128:(nt + 1) * 128],
                       rhs=xc[:, dt_, :],
                       start=(dt_ == 0), stop=(dt_ == NT - 1))
                nc.scalar.copy(graw[:, nt, qc * 512:(qc + 1) * 512], g_ps)

        def phase_p_tile(i):
            """projections + rmsnorm + rope (rstd deferred) for s-tile i."""
            c, quarter = i // 4, i % 4
            if quarter == 0 and c + 1 < NQC and (c + 1) not in xch:
                load_xchunk(c + 1)
            xt = xch[c][:, :, quarter * 128:(quarter + 1) * 128]

            # q and kv interleaved per dt_: the stationary operand (the x
            # tile) is shared, so the PE reloads weights once per dt_ instead
            # of twice and the ldweights stream stays off the critical path
            q_ps = psum.tile([128, 512], F32, tag="ps_qp", bufs=1, name=f"qps_{i}")
            kv_ps = psum.tile([128, 256], F32, tag="ps_to", bufs=2, name=f"kvps_{i}")
            for dt_ in range(NT):
                mm(q_ps, xt[:, dt_, :], rhs=wq_sb[:, dt_, :],
                   start=(dt_ == 0), stop=(dt_ == NT - 1))
                mm(kv_ps, xt[:, dt_, :], rhs=wkv_sb[:, dt_, :],
                   start=(dt_ == 0), stop=(dt_ == NT - 1))

            # ---- q: square+reduce (rstd later), scale, rope ----
            q3 = q_ps.rearrange("p (h e) -> p h e", e=64)
            sq = pwork.tile([128, 8, 64], BF16, tag="sq")
            nc.scalar.square(sq, q3)
            nc.vector.reduce_sum(ssq[:, i % 4, :], sq, axis=mybir.AxisListType.X)
            qn = pwork.tile([128, 8, 64], BF16, tag="qn")
            nc.vector.tensor_mul(qn.rearrange("p h e -> p (h e)"), q_ps, qsc_sb)
            rot = pwork.tile([128, 8, 64], BF16, tag="rot")
            nc.vector.tensor_scalar_mul(rot[:, :, 0:32], qn[:, :, 32:64], -1.0)
            nc.vector.tensor_copy(rot[:, :, 32:64], qn[:, :, 0:32])
            qf = qfc[i % 4]
            nc.vector.tensor_mul(qf, qn, _bc(cos_sb[:, i, :], 8, "mid"))
            nc.vector.tensor_mul(rot, rot, _bc(sin_sb[:, i, :], 8, "mid"))
            nc.vector.tensor_add(qf.rearrange("p h e -> p (h e)"),
                                 qf.rearrange("p h e -> p (h e)"),
                                 rot.rearrange("p h e -> p (h e)"))

            # ---- k ----
            k3 = kv_ps[:, 0:128].rearrange("p (h e) -> p h e", e=64)
            ksq = pwork.tile([128, 2, 64], BF16, tag="ksq")
            nc.scalar.square(ksq, k3)
            nc.vector.reduce_sum(ssk[:, i % 4, :], ksq, axis=mybir.AxisListType.X)
            kn = pwork.tile([128, 2, 64], BF16, tag="kn")
            nc.vector.tensor_mul(kn.rearrange("p h e -> p (h e)"),
                                 kv_ps[:, 0:128], ksc_sb)
            krot = pwork.tile([128, 2, 64], BF16, tag="krot")
            nc.vector.tensor_scalar_mul(krot[:, :, 0:32], kn[:, :, 32:64], -1.0)
            nc.vector.tensor_copy(krot[:, :, 32:64], kn[:, :, 0:32])
            kf = kfc[i % 4]
            nc.vector.tensor_mul(kf, kn, _bc(cos_sb[:, i, :], 2, "mid"))
            nc.vector.tensor_mul(krot, krot, _bc(sin_sb[:, i, :], 2, "mid"))
            nc.vector.tensor_add(kf.rearrange("p h e -> p (h e)"),
                                 kf.rearrange("p h e -> p (h e)"),
                                 krot.rearrange("p h e -> p (h e)"))

            # v into v_ext tiles (cast to bf16)
            nc.scalar.copy(vexta[:, i, 0:64], kv_ps[:, 128:192])
            nc.scalar.copy(vextb[:, i, 64:128], kv_ps[:, 192:256])

        def tail_gate(qc):
            """sigmoid on the chunk's gate slab (depends on the gate chains)"""
            gsl = graw[:, :, qc * 512:(qc + 1) * 512]
            nc.scalar.activation(gsl, gsl, AF.Sigmoid)

        def phase_p_chunk_tail(qc):
            """batched rsqrt + rstd application + transposes.

            Depends only on the chunk's four P tiles (not on the gate
            chains), so it is emitted as early as possible: the next chunk's
            attention pairs wait on the qT/kT transposes emitted here."""
            nc.scalar.activation(ssq, ssq, AF.Sqrt, bias=eps_sb, scale=1.0 / 64)
            nc.scalar.activation(ssk, ssk, AF.Sqrt, bias=eps_sb, scale=1.0 / 64)
            nc.vector.reciprocal(ssq, ssq)
            nc.vector.reciprocal(ssk, ssk)
            for u in range(4):
                i = qc * 4 + u
                qf, kf = qfc[u], kfc[u]
                nc.vector.tensor_mul(qf, qf, _bc(ssq[:, u, :], 64))
                nc.vector.tensor_mul(kf, kf, _bc(ssk[:, u, :], 64))
                qf2 = qf.rearrange("p h e -> p (h e)")
                for nt in range(4):
                    tp = psum.tile([128, 128], BF16, tag="ps_s", bufs=3,
                                   name=f"tp_{i}_{nt}")
                    nc.tensor.transpose(tp, qf2[:, nt * 128:(nt + 1) * 128],
                                        ident_sb)
                    nc.vector.tensor_copy(qT[:, nt, i * 128:(i + 1) * 128], tp)
                kf2 = kf.rearrange("p h e -> p (h e)")
                tpk = psum.tile([128, 128], BF16, tag="ps_s", bufs=3,
                                name=f"tpk_{i}")
                nc.tensor.transpose(tpk, kf2, ident_sb)
                nc.vector.tensor_copy(kT[:, i * 128:(i + 1) * 128], tpk)

        def a_pair(qc, nt, ctxg):
            """attention for head pair (g0 head nt, g1 head 4+nt) of chunk qc.

            Scores for the two heads are emitted back-to-back as disjoint
            PE row-group tiles (kT partitions 0-63 vs 64-127) into separate
            PSUM banks, so they stream through the array concurrently."""
            kts = [kt for kt in range(NT) if classes[qc][kt] != "skip"]
            ctxA = psum.tile([128, 512], F32, tag="ps_ctx", bufs=2,
                             name=f"ctxA_{qc}_{nt}")
            ctxB = psum.tile([128, 512], F32, tag="ps_ctx", bufs=2,
                             name=f"ctxB_{qc}_{nt}")

            def emit_score(j):
                kt = kts[j]
                cls = classes[qc][kt]
                lo = cls[1] if cls[0] == "tri" else 0
                ktw = kT[:, kt * 128:(kt + 1) * 128]
                qw = qT[:, nt, qc * 512 + lo:(qc + 1) * 512]
                sA = psum.tile([128, 512], F32, tag="ps_s", bufs=3,
                               name=f"sA_{qc}_{nt}_{kt}")
                sB = psum.tile([128, 512], F32, tag="ps_s", bufs=3,
                               name=f"sB_{qc}_{nt}_{kt}")
                mm(sA[:, lo:512], ktw[0:64, :], rhs=qw[0:64, :],
                   start=True, stop=True)
                mm(sB[:, lo:512], ktw[64:128, :], rhs=qw[64:128, :],
                   start=True, stop=True)
                # one sbuf tile for both heads: the tri/mask multiply then
                # covers the pair in a single strided DVE op
                eAB = awork.tile([128, 2, 512], BF16, tag="eT", bufs=3)
                nc.scalar.activation(eAB[:, 0, lo:512], sA[:, lo:512], AF.Exp)
                nc.scalar.activation(eAB[:, 1, lo:512], sB[:, lo:512], AF.Exp)
                if cls[0] == "tri":
                    nc.vector.tensor_mul(eAB[:, :, lo:lo + 128],
                                         eAB[:, :, lo:lo + 128],
                                         _bc(tri_sb[:, :], 2, "mid"))
                elif cls[0] == "mask":
                    nc.vector.tensor_mul(eAB, eAB,
                                         _bc(mask_sb[:, cls[1], :], 2, "mid"))
                return eAB, lo

            def emit_ctx(j, eAB, lo):
                last = (j == len(kts) - 1)
                mm(ctxA[0:65, lo:512], vexta[:, kts[j], :],
                   rhs=eAB[:, 0, lo:512], start=(j == 0), stop=last)
                mm(ctxB[:, lo:512], vextb[:, kts[j], :],
                   rhs=eAB[:, 1, lo:512], start=(j == 0), stop=last)

            # scores emitted one step ahead of the ctx accumulation so the
            # PE stream never head-of-line blocks on an exp in flight
            pend = None
            for j in range(len(kts)):
                cur = emit_score(j)
                if pend is not None:
                    emit_ctx(j - 1, *pend)
                pend = cur
            emit_ctx(len(kts) - 1, *pend)

            def normalize():
                # denomA at ctxA row 64; denomB at ctxB row 0 (vextb ones@0).
                # Two K=1 broadcast matmuls (disjoint row+col groups, same
                # bank) expand both reciprocals to [128,512]; one DVE mul
                # applies the full-width gate slab; both ctxg halves write
                # in-lane (ctxB data lives at rows 64-127).
                rstage = awork.tile([128, 512], BF16, tag="rstage", bufs=2)
                with nc.allow_low_precision(reason="bf16 softmax denom"):
                    nc.vector.reciprocal(rstage[64:65, :], ctxA[64:65, :])
                    nc.vector.reciprocal(rstage[0:1, :], ctxB[0:1, :])
                rb_ps = psum.tile([128, 512], F32, tag="ps_s", bufs=3,
                                  name=f"rbps_{qc}_{nt}")
                mm(rb_ps[0:64, :], ones_sb[64:65, :],
                   rhs=rstage[64:65, :], start=True, stop=True)
                mm(rb_ps[64:128, :], ones_sb[0:1, :],
                   rhs=rstage[0:1, :], start=True, stop=True)
                m1 = awork.tile([128, 512], BF16, tag="m1", bufs=2)
                nc.vector.tensor_mul(m1, rb_ps,
                                     graw[:, nt, qc * 512:(qc + 1) * 512])
                nc.vector.tensor_mul(ctxg[nt][0:64, :], ctxA[0:64, :],
                                     m1[0:64, :])
                nc.vector.tensor_mul(ctxg[nt][64:128, :], ctxB[64:128, :],
                                     m1[64:128, :])
            return normalize

        def a_outproj_ssub(qc, ctxg, ssub):
            """output projection for one 128-row slab of q-chunk qc."""
            srow = qc * 512 + ssub * 128
            ostage = awork.tile([128, D], BF16, tag="ostage", bufs=3)
            for dc in range(4):
                o_ps = psum.tile([128, 512], F32, tag="ps_to", bufs=2,
                                 name=f"ops_{qc}_{ssub}_{dc}")
                for nt in range(4):
                    mm(o_ps, ctxg[nt][:, ssub * 128:(ssub + 1) * 128],
                       rhs=wo_sb[:, nt, dc * 512:(dc + 1) * 512],
                       start=(nt == 0), stop=(nt == 3))
                # split the psum drains between ACT and DVE (copy lives in
                # every ACT table set, so no table reloads are triggered)
                if dc % 2 == (0 if qc == NQC - 1 else 1):
                    nc.scalar.copy(ostage[:, dc * 512:(dc + 1) * 512], o_ps)
                else:
                    nc.vector.tensor_copy(ostage[:, dc * 512:(dc + 1) * 512],
                                          o_ps)
            nc.sync.dma_start(out=y.ap()[srow:srow + 128, :], in_=ostage)

        # ======== main schedule: P chunk -> (tail) -> A chunk, interleaved ==
        ssq = singles.tile([128, 4, 8], F32)
        ssk = singles.tile([128, 4, 2], F32)
        qfc = [singles.tile([128, 8, 64], BF16, name=f"qfc{u}") for u in range(4)]
        kfc = [singles.tile([128, 2, 64], BF16, name=f"kfc{u}") for u in range(4)]

        # PE warm-up on memset data while the first weight/x DMAs stream in:
        # ~4us of array activity flips the HAM clock gate to 8/8 (2.4 GHz)
        # before the first real projection chain issues, and costs nothing
        # (the PE would be idle waiting on DMA anyway).
        warm_rhs = vextb[:, 0:8, 0:64]
        for w in range(12):
            warm_ps = psum.tile([64, 512], F32, tag="ps_s", bufs=3,
                                name=f"warm_{w}")
            mm(warm_ps, ones_sb, rhs=warm_rhs, start=True, stop=True)

        # A(qc) head-pairs interleaved with P tiles of chunk qc+1 and the
        # out-projection slabs of chunk qc-1, so the PE always has dense
        # projection matmuls to chew on while ACT runs the exps.  The gate
        # chunk + P tail of chunk qc+1 and two trailing out-projection slabs
        # land after the pairs: PE-dense work that fills the tail's
        # sigmoid/rsqrt/transpose window.
        for u in range(4):
            phase_p_tile(u)
        emit_g_chunk(0)
        phase_p_chunk_tail(0)
        tail_gate(0)
        prev = None  # (qc, ctxg) awaiting out-projection
        for qc in range(NQC):
            load_wo()
            ctxg = [awork.tile([128, 512], BF16, tag=f"ctxg{nt}",
                               name=f"ctxg{nt}_{qc}", bufs=2)
                    for nt in range(4)]
            for u in range(4):
                norm = a_pair(qc, u, ctxg)
                norm()
                if qc + 1 < NQC:
                    phase_p_tile((qc + 1) * 4 + u)
                if prev is not None and u == 2:
                    a_outproj_ssub(prev[0], prev[1], 0)
                if prev is not None and u == 3:
                    a_outproj_ssub(prev[0], prev[1], 1)
            # transpose tail emitted before the gate chains: the next
            # chunk's pairs wait on qT/kT, while the gate chains are
            # independent PE work that fills the tail's ACT/DVE window
            if qc + 1 < NQC:
                phase_p_chunk_tail(qc + 1)
                emit_g_chunk(qc + 1)
                tail_gate(qc + 1)
            if prev is not None:
                a_outproj_ssub(prev[0], prev[1], 2)
                a_outproj_ssub(prev[0], prev[1], 3)
            prev = (qc, ctxg)
        for ssub in range(4):
            a_outproj_ssub(prev[0], prev[1], ssub)

    nc.compile()
    return nc


# ======================== host-side runner =================================
_CACHE = {}


class _Runner:
    """Jitted sharded executable for a prebuilt Bass module, reusable."""

    def __init__(self, nc, n_cores):
        import jax
        import numpy as _np
        from jax.sharding import Mesh, PartitionSpec
        from jax.experimental.shard_map import shard_map
        from concourse.bass2jax import (_bass_exec_p, partition_id_tensor,
                                        install_neuronx_cc_hook)
        install_neuronx_cc_hook()
        self.jax = jax
        self.nc = nc
        self.n_cores = n_cores
        partition_name = (nc.partition_id_tensor.name
                          if nc.partition_id_tensor else None)
        in_names, out_names, out_avals = [], [], []
        for alloc in nc.m.functions[0].allocations:
            if not isinstance(alloc, mybir.MemoryLocationSet):
                continue
            name = alloc.memorylocations[0].name
            if alloc.kind == "ExternalInput":
                if name != partition_name:
                    in_names.append(name)
            elif alloc.kind == "ExternalOutput":
                out_names.append(name)
                out_avals.append(jax.core.ShapedArray(
                    tuple(alloc.tensor_shape), mybir.dt.np(alloc.dtype)))
        self.in_names, self.out_names, self.out_avals = in_names, out_names, out_avals
        all_in = list(in_names) + list(out_names)
        if partition_name is not None:
            all_in.append(partition_name)
        self._partition_name = partition_name
        self._all_in = all_in
        self._dbg_name = nc.dbg_addr.name if nc.dbg_addr is not None else None

        devices = jax.devices()[:n_cores]
        self.mesh = Mesh(_np.asarray(devices), ("core",))
        self.fn = None
        self.dev_in = None

    def _compile_fast(self):
        """AOT-compile the shard_map body on the effect-free C++ fast-dispatch
        path (saves ~700us/exec of python dispatch overhead)."""
        import jax
        from jax.sharding import PartitionSpec
        from jax.experimental.shard_map import shard_map
        import concourse.bass2jax as b2j
        nc = self.nc
        partition_name = self._partition_name
        all_in, out_names, out_avals = self._all_in, self.out_names, self.out_avals

        def _body(*args):
            operands = list(args)
            if partition_name is not None:
                operands.append(b2j.partition_id_tensor())
            outs = b2j._bass_exec_p.bind(
                *operands, out_avals=tuple(out_avals), in_names=tuple(all_in),
                out_names=tuple(out_names), lowering_input_output_aliases=(),
                sim_require_finite=True, sim_require_nnan=True, nc=nc)
            return tuple(outs)

        n = len(self.in_names) + len(out_names)

        def compile_fn():
            f = jax.jit(shard_map(
                _body, mesh=self.mesh,
                in_specs=(PartitionSpec("core"),) * n,
                out_specs=(PartitionSpec("core"),) * len(out_names),
                check_rep=False))
            return f.lower(*self.dev_in).compile()

        try:
            self.fn = b2j.fast_dispatch_compile(compile_fn)
        except Exception:
            f = jax.jit(shard_map(
                _body, mesh=self.mesh,
                in_specs=(PartitionSpec("core"),) * n,
                out_specs=(PartitionSpec("core"),) * len(out_names),
                check_rep=False))
            self.fn = f

    def prepare(self, in_maps):
        import numpy as _np
        from jax.sharding import NamedSharding, PartitionSpec
        if self._dbg_name is not None:
            in_maps = [{**m, self._dbg_name: _np.zeros((1, 2), _np.uint32)}
                       for m in in_maps]
        concat = [_np.concatenate([_np.asarray(in_maps[c][n])
                                   for c in range(self.n_cores)], axis=0)
                  for n in self.in_names]
        # zero output buffers: device-resident, NOT donated, reused each run.
        # Valid because the kernel writes every element of its outputs.
        concat += [_np.zeros((self.n_cores * av.shape[0], *av.shape[1:]),
                             av.dtype) for av in self.out_avals]
        shard = NamedSharding(self.mesh, PartitionSpec("core"))
        self.dev_in = [self.jax.device_put(a, shard) for a in concat]
        if self.fn is None:
            self._compile_fast()
        return self

    def run(self):
        return self.jax.block_until_ready(self.fn(*self.dev_in))

    def results(self, outs):
        import numpy as _np
        res = []
        for c in range(self.n_cores):
            d = {}
            for i, name in enumerate(self.out_names):
                full = _np.asarray(outs[i])
                d[name] = full.reshape(self.n_cores, *self.out_avals[i].shape)[c]
            res.append(d)
        return res


def make_runner(nc, n_cores):
    return _Runner(nc, n_cores)


def _prep_core_inputs(inputs, b, t, shared):
    x = inputs["x"]
    import ml_dtypes
    bf = ml_dtypes.bfloat16

    if ("xT", b) not in shared:
        shared[("xT", b)] = np.ascontiguousarray(np.asarray(x[b]).T).astype(bf)
    if ("w", t) not in shared:
        Wq, Wk, Wv, Wg, Wo = (np.asarray(inputs[k])
                              for k in ("Wq", "Wk", "Wv", "Wg", "Wo"))
        heads = [8 * t + p for p in _PERM]
        qcols = np.concatenate([np.arange(h * 64, (h + 1) * 64) for h in heads])
        groups = [2 * t, 2 * t + 1]
        kcols = np.concatenate([np.arange(g * 64, (g + 1) * 64) for g in groups])
        shared[("w", t)] = {
            "wq": np.ascontiguousarray(Wq[:, qcols]).astype(bf),
            "wkv": np.ascontiguousarray(
                np.concatenate([Wk[:, kcols], Wv[:, kcols]], axis=1)).astype(bf),
            "wg": np.ascontiguousarray(Wg[:, qcols]).astype(bf),
            "wo": np.ascontiguousarray(Wo[qcols, :]).astype(bf),
        }
    if "const" not in shared:
        q_scale, k_scale = np.asarray(inputs["q_scale"]), np.asarray(inputs["k_scale"])
        cos, sin = np.asarray(inputs["cos"]), np.asarray(inputs["sin"])
        scaling = float(HD) ** -0.5
        tri = (np.arange(128)[:, None] <= np.arange(128)[None, :])
        shared["const"] = {
            "cosd": cos.astype(bf), "sind": sin.astype(bf),
            "qsc": np.broadcast_to(
                np.tile((1.0 + q_scale) * scaling, 8)[None, :],
                (128, 512)).astype(np.float32).copy(),
            "ksc": np.broadcast_to(
                np.tile(1.0 + k_scale, 2)[None, :],
                (128, 128)).astype(np.float32).copy(),
            "trit": tri.astype(bf),
            "ident": np.eye(128, dtype=np.float32).astype(bf),
        }
    return {"xT": shared[("xT", b)], **shared[("w", t)], **shared["const"]}


def kernel(**inputs):
    mask = np.asarray(inputs["mask"])
    classes, tiles = classify_mask(mask)
    key = mask.tobytes()
    if key not in _CACHE:
        nc = build_program(classes, len(tiles))
        _CACHE[key] = (nc, make_runner(nc, NCORES))
    nc, runner = _CACHE[key]

    import ml_dtypes
    mask_arr = (np.stack(tiles).astype(ml_dtypes.bfloat16) if tiles else None)
    shared = {}
    in_maps = []
    for c in range(NCORES):
        m = _prep_core_inputs(inputs, c // 4, c % 4, shared)
        if mask_arr is not None:
            m["maskt"] = mask_arr
        in_maps.append(m)

    runner.prepare(in_maps)
    outs = runner.run()
    res = runner.results(outs)
    out = np.zeros((B, S, D), np.float32)
    for c in range(NCORES):
        out[c // 4] += res[c]["y"].astype(np.float32)
    return out.astype(np.asarray(inputs["x"]).dtype)


# revision 17
# speedup vs baseline: 1.0127x; 1.0030x over previous
"""GQA kernel for 8x TRN2 NeuronCores (Bass/Tile), DP2 x TP4 sharding.

Layout strategy (per core; batch b = core//4, shard t = core%4):
  - x fed transposed (feature-major) xT [D, S]; projections emit token-major
    q/k/v and feature-major gate^T via PE matmuls.  x is staged in 512-token
    chunks so the gate chains run at N=512 (4 chains/chunk instead of 16
    N=128 chains -- 4x fewer PE instructions for the same cycles).
  - rmsnorm+rope token-major (free-dim reductions), then PE-transpose q,k to
    feature-major for attention. rstd is applied AFTER rope (it commutes: a
    per-(token,head) scalar) via ACT Rsqrt + one broadcast-AP DVE mul per
    tile; rope cos/sin muls are likewise single broadcast-AP ops per tile.
  - attention runs on head PAIRS (g0 head on kT/qT partitions 0-63, g1 head
    on 64-127): the two scores^T matmuls are emitted back-to-back with
    disjoint PE row-groups (tile_position (0,0) / (64,0)) into separate PSUM
    banks, so the array streams both heads concurrently (~2x scores).
  - ctx^T accumulated feature-major. Group 0 uses v_ext=[v|ones] (M=65,
    denom at psum row 64); group 1 uses v_ext=[ones|0|v] (M=128, denom at
    row 0, data at rows 64-127). Two K=1 broadcast matmuls (concurrent
    row+col tiles) expand both reciprocals into one [128,512] psum, which
    multiplies the full-width gate slab in ONE DVE op; both halves of ctxg
    are then written in-lane -- no cross-partition DMAs in normalize at all.
  - out projection token-major with ctxg stationary; y emitted bf16; partial
    [S, D] outputs summed across the 4 TP shards on host.
Local head order is interleaved (0,4,1,5,2,6,3,7) so transposed q tiles put a
g0 head on partitions 0-63 and a g1 head on 64-127, matching kT/gate/Wo
layouts without any cross-partition moves.
"""
import sys

if "/opt/trn_rl_repo" not in sys.path:
    sys.path.insert(0, "/opt/trn_rl_repo")

import numpy as np

import concourse.bass as bass
import concourse.mybir as mybir
import concourse.tile as tile
from concourse import bacc

B, S, D = 2, 2048, 2048
H, G, HD = 32, 8, 64
EPS = 1e-6
NCORES = 8
NT = S // 128          # 16 s-tiles
NQC = S // 512         # 4 q-chunks
F32 = mybir.dt.float32
BF16 = mybir.dt.bfloat16

_PERM = [0, 4, 1, 5, 2, 6, 3, 7]  # local head order (token-major col blocks)


def _bc(ap, n, where="last"):
    """stride-0 broadcast dim appended (or inserted after partition dim)."""
    if where == "last":
        return bass.AP(tensor=ap.tensor, offset=ap.offset, ap=[*ap.ap, [0, n]])
    return bass.AP(tensor=ap.tensor, offset=ap.offset,
                   ap=[ap.ap[0], [0, n], *ap.ap[1:]])


def classify_mask(mask):
    """Per (qc, kt) block class for scores^T blocks.

    Returns (classes, tiles) where classes[qc][kt] is one of
      'skip'            -- fully masked block
      ('clean',)        -- no masking
      ('tri', lo)       -- causal diagonal block: cols < lo fully masked,
                           cols [lo, lo+128) lower-triangle, rest clean
      ('mask', idx)     -- general: multiply full width by tiles[idx]
    """
    classes = []
    tiles = []
    keyidx = {}
    q_loc = np.arange(512)[:, None]
    k_loc = np.arange(128)[None, :]
    for qc in range(NQC):
        row = []
        for kt in range(NT):
            sub = mask[qc * 512:(qc + 1) * 512, kt * 128:(kt + 1) * 128]
            if sub.all():
                row.append("skip")
            elif not sub.any():
                row.append(("clean",))
            else:
                lo = kt * 128 - qc * 512
                if 0 <= lo <= 384:
                    expect = (q_loc - lo) < k_loc  # True = masked
                    if np.array_equal(sub, expect):
                        row.append(("tri", lo))
                        continue
                t = (~sub.T).astype(np.float32)  # [128k, 512q] 1=keep
                key = t.tobytes()
                if key not in keyidx:
                    keyidx[key] = len(tiles)
                    tiles.append(t)
                row.append(("mask", keyidx[key]))
        classes.append(row)
    return classes, tiles


def build_program(classes, n_masks):
    nc = bacc.Bacc("TRN2", target_bir_lowering=False, debug=False)

    def mm(out, lhsT, rhs, start, stop):
        nc.tensor.matmul(out, lhsT=lhsT, rhs=rhs, start=start, stop=stop)

    xT = nc.dram_tensor("xT", [D, S], BF16, kind="ExternalInput")
    wq = nc.dram_tensor("wq", [D, 512], BF16, kind="ExternalInput")
    wkv = nc.dram_tensor("wkv", [D, 256], BF16, kind="ExternalInput")
    wg = nc.dram_tensor("wg", [D, 512], BF16, kind="ExternalInput")
    wo = nc.dram_tensor("wo", [512, D], BF16, kind="ExternalInput")
    cosd = nc.dram_tensor("cosd", [S, HD], BF16, kind="ExternalInput")
    sind = nc.dram_tensor("sind", [S, HD], BF16, kind="ExternalInput")
    qsc = nc.dram_tensor("qsc", [128, 512], F32, kind="ExternalInput")
    ksc = nc.dram_tensor("ksc", [128, 128], F32, kind="ExternalInput")
    trit = nc.dram_tensor("trit", [128, 128], BF16, kind="ExternalInput")
    ident_in = nc.dram_tensor("ident", [128, 128], BF16, kind="ExternalInput")
    if n_masks:
        maskt = nc.dram_tensor("maskt", [n_masks, 128, 512], BF16,
                               kind="ExternalInput")
    y = nc.dram_tensor("y", [S, D], BF16, kind="ExternalOutput")

    AF = mybir.ActivationFunctionType
    from contextlib import ExitStack
    with tile.TileContext(nc) as tc, ExitStack() as es:
        singles = es.enter_context(tc.tile_pool(name="singles", bufs=1))
        xpool = es.enter_context(tc.tile_pool(name="xpool", bufs=2))
        pwork = es.enter_context(tc.tile_pool(name="pwork", bufs=2))
        psum = es.enter_context(tc.tile_pool(name="psum", bufs=1, space="PSUM"))
        awork = es.enter_context(tc.tile_pool(name="awork", bufs=3, space="SBUF"))

        # ---- resident constants / weights; order = startup DMA priority ----
        # wq/wkv/x arrive in interleaved plane-groups so the first (q,kv)
        # chain can start after ~1/4 of the weight bytes instead of all
        wq_sb = singles.tile([128, NT, 512], BF16)
        wq_r = wq.ap().rearrange("(a p) n -> p a n", p=128)
        wkv_sb = singles.tile([128, NT, 256], BF16)
        wkv_r = wkv.ap().rearrange("(a p) n -> p a n", p=128)

        xch = {}

        def load_xchunk(c, half=None):
            if c in xch:
                t = xch[c]
            else:
                t = xpool.tile([128, NT, 512], BF16, tag="xch", name=f"xch_{c}")
                xch[c] = t
            halves = (0, 1) if half is None else (half,)
            for k in halves:
                nc.sync.dma_start(
                    out=t[:, :, k * 256:(k + 1) * 256],
                    in_=xT.ap()[:, c * 512 + k * 256:c * 512 + (k + 1) * 256]
                    .rearrange("(a p) m -> p a m", p=128))

        nc.sync.dma_start(out=wq_sb[:, 0:8, :], in_=wq_r[:, 0:8, :])
        nc.sync.dma_start(out=wkv_sb[:, 0:8, :], in_=wkv_r[:, 0:8, :])
        load_xchunk(0, half=0)
        nc.sync.dma_start(out=wq_sb[:, 8:16, :], in_=wq_r[:, 8:16, :])
        nc.sync.dma_start(out=wkv_sb[:, 8:16, :], in_=wkv_r[:, 8:16, :])
        qsc_sb = singles.tile([128, 512], F32)
        nc.sync.dma_start(out=qsc_sb, in_=qsc.ap())
        ksc_sb = singles.tile([128, 128], F32)
        nc.sync.dma_start(out=ksc_sb, in_=ksc.ap())
        load_xchunk(0, half=1)
        cos_sb = singles.tile([128, NT, HD], BF16)
        nc.sync.dma_start(out=cos_sb, in_=cosd.ap().rearrange("(a p) n -> p a n", p=128))
        sin_sb = singles.tile([128, NT, HD], BF16)
        nc.sync.dma_start(out=sin_sb, in_=sind.ap().rearrange("(a p) n -> p a n", p=128))
        ident_sb = singles.tile([128, 128], BF16)
        nc.sync.dma_start(out=ident_sb, in_=ident_in.ap())
        tri_sb = singles.tile([128, 128], BF16)
        nc.sync.dma_start(out=tri_sb, in_=trit.ap())
        wg_sb = singles.tile([128, NT, 512], BF16)
        nc.sync.dma_start(out=wg_sb, in_=wg.ap().rearrange("(a p) n -> p a n", p=128))
        if n_masks:
            mask_sb = singles.tile([128, n_masks, 512], BF16)
            nc.sync.dma_start(out=mask_sb,
                              in_=maskt.ap().rearrange("a p n -> p a n"))
        # wo load deferred (emitted before the first out-projection) so it
        # does not block the x-chunk streaming DMAs at startup
        wo_sb = singles.tile([128, 4, D], BF16)
        wo_loaded = []

        def load_wo():
            if not wo_loaded:
                nc.sync.dma_start(
                    out=wo_sb, in_=wo.ap().rearrange("(a p) n -> p a n", p=128))
                wo_loaded.append(True)

        qT = singles.tile([128, 4, S], BF16)       # head nt @0-63, 4+nt @64-127
        kT = singles.tile([128, S], BF16)          # group0 @0-63, group1 @64-127
        vexta = singles.tile([128, NT, 65], BF16)   # [v(64) | ones]
        nc.vector.memset(vexta[:, :, 64], 1.0)
        vextb = singles.tile([128, NT, 128], BF16)  # [ones | 0*63 | v(64)]
        nc.vector.memset(vextb[:, :, 0:64], 0.0)
        nc.vector.memset(vextb[:, :, 0], 1.0)
        eps_sb = singles.tile([128, 1], F32)
        nc.vector.memset(eps_sb, float(EPS))
        ones_sb = singles.tile([128, 64], BF16)
        nc.vector.memset(ones_sb, 1.0)
        graw = singles.tile([128, 4, S], BF16)     # gate^T; sigmoid in-place

        def emit_g_chunk(qc):
            xc = xch[qc]
            for nt in range(4):
                g_ps = psum.tile([128, 512], F32, tag="ps_to", bufs=2,
                                 name=f"gps_{qc}_{nt}")
                for dt_ in range(NT):
                    mm(g_ps, wg_sb[:, dt_, nt * 128:(nt + 1) * 128],
                       rhs=xc[:, dt_, :],
                       start=(dt_ == 0), stop=(dt_ == NT - 1))
                nc.scalar.copy(graw[:, nt, qc * 512:(qc + 1) * 512], g_ps)

        def phase_p_tile(i):
            """projections + rmsnorm + rope (rstd deferred) for s-tile i."""
            c, quarter = i // 4, i % 4
            if quarter == 0 and c + 1 < NQC and (c + 1) not in xch:
                load_xchunk(c + 1)
            xt = xch[c][:, :, quarter * 128:(quarter + 1) * 128]

            # q and kv interleaved per dt_: the stationary operand (the x
            # tile) is shared, so the PE reloads weights once per dt_ instead
            # of twice and the ldweights stream stays off the critical path
            q_ps = psum.tile([128, 512], F32, tag="ps_qp", bufs=1, name=f"qps_{i}")
            kv_ps = psum.tile([128, 256], F32, tag="ps_to", bufs=2, name=f"kvps_{i}")
            for dt_ in range(NT):
                mm(q_ps, xt[:, dt_, :], rhs=wq_sb[:, dt_, :],
                   start=(dt_ == 0), stop=(dt_ == NT - 1))
                mm(kv_ps, xt[:, dt_, :], rhs=wkv_sb[:, dt_, :],
                   start=(dt_ == 0), stop=(dt_ == NT - 1))

            # ---- q: square+reduce (rstd later), scale, rope ----
            q3 = q_ps.rearrange("p (h e) -> p h e", e=64)
            sq = pwork.tile([128, 8, 64], BF16, tag="sq")
            nc.scalar.square(sq, q3)
            nc.vector.reduce_sum(ssq[:, i % 4, :], sq, axis=mybir.AxisListType.X)
            qn = pwork.tile([128, 8, 64], BF16, tag="qn")
            nc.vector.tensor_mul(qn.rearrange("p h e -> p (h e)"), q_ps, qsc_sb)
            rot = pwork.tile([128, 8, 64], BF16, tag="rot")
            nc.vector.tensor_scalar_mul(rot[:, :, 0:32], qn[:, :, 32:64], -1.0)
            nc.vector.tensor_copy(rot[:, :, 32:64], qn[:, :, 0:32])
            qf = qfc[i % 4]
            nc.vector.tensor_mul(qf, qn, _bc(cos_sb[:, i, :], 8, "mid"))
            nc.vector.tensor_mul(rot, rot, _bc(sin_sb[:, i, :], 8, "mid"))
            nc.vector.tensor_add(qf.rearrange("p h e -> p (h e)"),
                                 qf.rearrange("p h e -> p (h e)"),
                                 rot.rearrange("p h e -> p (h e)"))

            # ---- k ----
            k3 = kv_ps[:, 0:128].rearrange("p (h e) -> p h e", e=64)
            ksq = pwork.tile([128, 2, 64], BF16, tag="ksq")
            nc.scalar.square(ksq, k3)
            nc.vector.reduce_sum(ssk[:, i % 4, :], ksq, axis=mybir.AxisListType.X)
            kn = pwork.tile([128, 2, 64], BF16, tag="kn")
            nc.vector.tensor_mul(kn.rearrange("p h e -> p (h e)"),
                                 kv_ps[:, 0:128], ksc_sb)
            krot = pwork.tile([128, 2, 64], BF16, tag="krot")
            nc.vector.tensor_scalar_mul(krot[:, :, 0:32], kn[:, :, 32:64], -1.0)
            nc.vector.tensor_copy(krot[:, :, 32:64], kn[:, :, 0:32])
            kf = kfc[i % 4]
            nc.vector.tensor_mul(kf, kn, _bc(cos_sb[:, i, :], 2, "mid"))
            nc.vector.tensor_mul(krot, krot, _bc(sin_sb[:, i, :], 2, "mid"))
            nc.vector.tensor_add(kf.rearrange("p h e -> p (h e)"),
                                 kf.rearrange("p h e -> p (h e)"),
                                 krot.rearrange("p h e -> p (h e)"))

            # v into v_ext tiles (cast to bf16)
            nc.scalar.copy(vexta[:, i, 0:64], kv_ps[:, 128:192])
            nc.scalar.copy(vextb[:, i, 64:128], kv_ps[:, 192:256])

        def tail_gate(qc):
            """sigmoid on the chunk's gate slab (depends on the gate chains)"""
            gsl = graw[:, :, qc * 512:(qc + 1) * 512]
            nc.scalar.activation(gsl, gsl, AF.Sigmoid)

        def phase_p_chunk_tail(qc):
            """batched rsqrt + rstd application + transposes.

            Depends only on the chunk's four P tiles (not on the gate
            chains), so it is emitted as early as possible: the next chunk's
            attention pairs wait on the qT/kT transposes emitted here."""
            nc.scalar.activation(ssq, ssq, AF.Sqrt, bias=eps_sb, scale=1.0 / 64)
            nc.scalar.activation(ssk, ssk, AF.Sqrt, bias=eps_sb, scale=1.0 / 64)
            nc.vector.reciprocal(ssq, ssq)
            nc.vector.reciprocal(ssk, ssk)
            for u in range(4):
                i = qc * 4 + u
                qf, kf = qfc[u], kfc[u]
                nc.vector.tensor_mul(qf, qf, _bc(ssq[:, u, :], 64))
                nc.vector.tensor_mul(kf, kf, _bc(ssk[:, u, :], 64))
                qf2 = qf.rearrange("p h e -> p (h e)")
                for nt in range(4):
                    tp = psum.tile([128, 128], BF16, tag="ps_s", bufs=3,
                                   name=f"tp_{i}_{nt}")
                    nc.tensor.transpose(tp, qf2[:, nt * 128:(nt + 1) * 128],
                                        ident_sb)
                    nc.vector.tensor_copy(qT[:, nt, i * 128:(i + 1) * 128], tp)
                kf2 = kf.rearrange("p h e -> p (h e)")
                tpk = psum.tile([128, 128], BF16, tag="ps_s", bufs=3,
                                name=f"tpk_{i}")
                nc.tensor.transpose(tpk, kf2, ident_sb)
                nc.vector.tensor_copy(kT[:, i * 128:(i + 1) * 128], tpk)

        def a_pair(qc, nt, ctxg):
            """attention for head pair (g0 head nt, g1 head 4+nt) of chunk qc.

            Scores for the two heads are emitted back-to-back as disjoint
            PE row-group tiles (kT partitions 0-63 vs 64-127) into separate
            PSUM banks, so they stream through the array concurrently."""
            kts = [kt for kt in range(NT) if classes[qc][kt] != "skip"]
            ctxA = psum.tile([128, 512], F32, tag="ps_ctx", bufs=2,
                             name=f"ctxA_{qc}_{nt}")
            ctxB = psum.tile([128, 512], F32, tag="ps_ctx", bufs=2,
                             name=f"ctxB_{qc}_{nt}")

            def emit_score(j):
                kt = kts[j]
                cls = classes[qc][kt]
                lo = cls[1] if cls[0] == "tri" else 0
                ktw = kT[:, kt * 128:(kt + 1) * 128]
                qw = qT[:, nt, qc * 512 + lo:(qc + 1) * 512]
                sA = psum.tile([128, 512], F32, tag="ps_s", bufs=3,
                               name=f"sA_{qc}_{nt}_{kt}")
                sB = psum.tile([128, 512], F32, tag="ps_s", bufs=3,
                               name=f"sB_{qc}_{nt}_{kt}")
                mm(sA[:, lo:512], ktw[0:64, :], rhs=qw[0:64, :],
                   start=True, stop=True)
                mm(sB[:, lo:512], ktw[64:128, :], rhs=qw[64:128, :],
                   start=True, stop=True)
                # one sbuf tile for both heads: the tri/mask multiply then
                # covers the pair in a single strided DVE op
                eAB = awork.tile([128, 2, 512], BF16, tag="eT", bufs=3)
                nc.scalar.activation(eAB[:, 0, lo:512], sA[:, lo:512], AF.Exp)
                nc.scalar.activation(eAB[:, 1, lo:512], sB[:, lo:512], AF.Exp)
                if cls[0] == "tri":
                    nc.vector.tensor_mul(eAB[:, :, lo:lo + 128],
                                         eAB[:, :, lo:lo + 128],
                                         _bc(tri_sb[:, :], 2, "mid"))
                elif cls[0] == "mask":
                    nc.vector.tensor_mul(eAB, eAB,
                                         _bc(mask_sb[:, cls[1], :], 2, "mid"))
                return eAB, lo

            def emit_ctx(j, eAB, lo):
                last = (j == len(kts) - 1)
                mm(ctxA[0:65, lo:512], vexta[:, kts[j], :],
                   rhs=eAB[:, 0, lo:512], start=(j == 0), stop=last)
                mm(ctxB[:, lo:512], vextb[:, kts[j], :],
                   rhs=eAB[:, 1, lo:512], start=(j == 0), stop=last)

            # scores emitted one step ahead of the ctx accumulation so the
            # PE stream never head-of-line blocks on an exp in flight
            pend = None
            for j in range(len(kts)):
                cur = emit_score(j)
                if pend is not None:
                    emit_ctx(j - 1, *pend)
                pend = cur
            emit_ctx(len(kts) - 1, *pend)

            def normalize():
                # denomA at ctxA row 64; denomB at ctxB row 0 (vextb ones@0).
                # Two K=1 broadcast matmuls (disjoint row+col groups, same
                # bank) expand both reciprocals to [128,512]; one DVE mul
                # applies the full-width gate slab; both ctxg halves write
                # in-lane (ctxB data lives at rows 64-127).
                rstage = awork.tile([128, 512], BF16, tag="rstage", bufs=2)
                with nc.allow_low_precision(reason="bf16 softmax denom"):
                    nc.vector.reciprocal(rstage[64:65, :], ctxA[64:65, :])
                    nc.vector.reciprocal(rstage[0:1, :], ctxB[0:1, :])
                rb_ps = psum.tile([128, 512], F32, tag="ps_s", bufs=3,
                                  name=f"rbps_{qc}_{nt}")
                mm(rb_ps[0:64, :], ones_sb[64:65, :],
                   rhs=rstage[64:65, :], start=True, stop=True)
                mm(rb_ps[64:128, :], ones_sb[0:1, :],
                   rhs=rstage[0:1, :], start=True, stop=True)
                m1 = awork.tile([128, 512], BF16, tag="m1", bufs=2)
                nc.vector.tensor_mul(m1, rb_ps,
                                     graw[:, nt, qc * 512:(qc + 1) * 512])
                nc.vector.tensor_mul(ctxg[nt][0:64, :], ctxA[0:64, :],
                                     m1[0:64, :])
                nc.vector.tensor_mul(ctxg[nt][64:128, :], ctxB[64:128, :],
                                     m1[64:128, :])
            return normalize

        def a_outproj_ssub(qc, ctxg, ssub):
            """output projection for one 128-row slab of q-chunk qc."""
            srow = qc * 512 + ssub * 128
            ostage = awork.tile([128, D], BF16, tag="ostage", bufs=3)
            for dc in range(4):
                o_ps = psum.tile([128, 512], F32, tag="ps_to", bufs=2,
                                 name=f"ops_{qc}_{ssub}_{dc}")
                for nt in range(4):
                    mm(o_ps, ctxg[nt][:, ssub * 128:(ssub + 1) * 128],
                       rhs=wo_sb[:, nt, dc * 512:(dc + 1) * 512],
                       start=(nt == 0), stop=(nt == 3))
                # split the psum drains between ACT and DVE (copy lives in
                # every ACT table set, so no table reloads are triggered)
                if dc % 2 == (0 if qc == NQC - 1 else 1):
                    nc.scalar.copy(ostage[:, dc * 512:(dc + 1) * 512], o_ps)
                else:
                    nc.vector.tensor_copy(ostage[:, dc * 512:(dc + 1) * 512],
                                          o_ps)
            nc.sync.dma_start(out=y.ap()[srow:srow + 128, :], in_=ostage)

        # ======== main schedule: P chunk -> (tail) -> A chunk, interleaved ==
        ssq = singles.tile([128, 4, 8], F32)
        ssk = singles.tile([128, 4, 2], F32)
        qfc = [singles.tile([128, 8, 64], BF16, name=f"qfc{u}") for u in range(4)]
        kfc = [singles.tile([128, 2, 64], BF16, name=f"kfc{u}") for u in range(4)]

        # PE warm-up on memset data while the first weight/x DMAs stream in:
        # ~4us of array activity flips the HAM clock gate to 8/8 (2.4 GHz)
        # before the first real projection chain issues, and costs nothing
        # (the PE would be idle waiting on DMA anyway).
        warm_rhs = vextb[:, 0:8, 0:64]
        for w in range(12):
            warm_ps = psum.tile([64, 512], F32, tag="ps_s", bufs=3,
                                name=f"warm_{w}")
            mm(warm_ps, ones_sb, rhs=warm_rhs, start=True, stop=True)

        # A(qc) head-pairs interleaved with P tiles of chunk qc+1 and the
        # out-projection slabs of chunk qc-1, so the PE always has dense
        # projection matmuls to chew on while ACT runs the exps.  The gate
        # chunk + P tail of chunk qc+1 and two trailing out-projection slabs
        # land after the pairs: PE-dense work that fills the tail's
        # sigmoid/rsqrt/transpose window.
        for u in range(4):
            phase_p_tile(u)
        emit_g_chunk(0)
        phase_p_chunk_tail(0)
        tail_gate(0)
        prev = None  # (qc, ctxg) awaiting out-projection
        for qc in range(NQC):
            load_wo()
            ctxg = [awork.tile([128, 512], BF16, tag=f"ctxg{nt}",
                               name=f"ctxg{nt}_{qc}", bufs=2)
                    for nt in range(4)]
            for u in range(4):
                norm = a_pair(qc, u, ctxg)
                norm()
                if qc + 1 < NQC:
                    phase_p_tile((qc + 1) * 4 + u)
                if prev is not None and u == 2:
                    a_outproj_ssub(prev[0], prev[1], 0)
                if prev is not None and u == 3:
                    a_outproj_ssub(prev[0], prev[1], 1)
            if qc + 1 < NQC:
                emit_g_chunk(qc + 1)
                tail_gate(qc + 1)
                phase_p_chunk_tail(qc + 1)
            if prev is not None:
                a_outproj_ssub(prev[0], prev[1], 2)
                a_outproj_ssub(prev[0], prev[1], 3)
            prev = (qc, ctxg)
        for ssub in range(4):
            a_outproj_ssub(prev[0], prev[1], ssub)

    nc.compile()
    return nc


# ======================== host-side runner =================================
_CACHE = {}


class _Runner:
    """Jitted sharded executable for a prebuilt Bass module, reusable."""

    def __init__(self, nc, n_cores):
        import jax
        import numpy as _np
        from jax.sharding import Mesh, PartitionSpec
        from jax.experimental.shard_map import shard_map
        from concourse.bass2jax import (_bass_exec_p, partition_id_tensor,
                                        install_neuronx_cc_hook)
        install_neuronx_cc_hook()
        self.jax = jax
        self.nc = nc
        self.n_cores = n_cores
        partition_name = (nc.partition_id_tensor.name
                          if nc.partition_id_tensor else None)
        in_names, out_names, out_avals = [], [], []
        for alloc in nc.m.functions[0].allocations:
            if not isinstance(alloc, mybir.MemoryLocationSet):
                continue
            name = alloc.memorylocations[0].name
            if alloc.kind == "ExternalInput":
                if name != partition_name:
                    in_names.append(name)
            elif alloc.kind == "ExternalOutput":
                out_names.append(name)
                out_avals.append(jax.core.ShapedArray(
                    tuple(alloc.tensor_shape), mybir.dt.np(alloc.dtype)))
        self.in_names, self.out_names, self.out_avals = in_names, out_names, out_avals
        all_in = list(in_names) + list(out_names)
        if partition_name is not None:
            all_in.append(partition_name)
        self._partition_name = partition_name
        self._all_in = all_in
        self._dbg_name = nc.dbg_addr.name if nc.dbg_addr is not None else None

        devices = jax.devices()[:n_cores]
        self.mesh = Mesh(_np.asarray(devices), ("core",))
        self.fn = None
        self.dev_in = None

    def _compile_fast(self):
        """AOT-compile the shard_map body on the effect-free C++ fast-dispatch
        path (saves ~700us/exec of python dispatch overhead)."""
        import jax
        from jax.sharding import PartitionSpec
        from jax.experimental.shard_map import shard_map
        import concourse.bass2jax as b2j
        nc = self.nc
        partition_name = self._partition_name
        all_in, out_names, out_avals = self._all_in, self.out_names, self.out_avals

        def _body(*args):
            operands = list(args)
            if partition_name is not None:
                operands.append(b2j.partition_id_tensor())
            outs = b2j._bass_exec_p.bind(
                *operands, out_avals=tuple(out_avals), in_names=tuple(all_in),
                out_names=tuple(out_names), lowering_input_output_aliases=(),
                sim_require_finite=True, sim_require_nnan=True, nc=nc)
            return tuple(outs)

        n = len(self.in_names) + len(out_names)

        def compile_fn():
            f = jax.jit(shard_map(
                _body, mesh=self.mesh,
                in_specs=(PartitionSpec("core"),) * n,
                out_specs=(PartitionSpec("core"),) * len(out_names),
                check_rep=False))
            return f.lower(*self.dev_in).compile()

        try:
            self.fn = b2j.fast_dispatch_compile(compile_fn)
        except Exception:
            f = jax.jit(shard_map(
                _body, mesh=self.mesh,
                in_specs=(PartitionSpec("core"),) * n,
                out_specs=(PartitionSpec("core"),) * len(out_names),
                check_rep=False))
            self.fn = f

    def prepare(self, in_maps):
        import numpy as _np
        from jax.sharding import NamedSharding, PartitionSpec
        if self._dbg_name is not None:
            in_maps = [{**m, self._dbg_name: _np.zeros((1, 2), _np.uint32)}
                       for m in in_maps]
        concat = [_np.concatenate([_np.asarray(in_maps[c][n])
                                   for c in range(self.n_cores)], axis=0)
                  for n in self.in_names]
        # zero output buffers: device-resident, NOT donated, reused each run.
        # Valid because the kernel writes every element of its outputs.
        concat += [_np.zeros((self.n_cores * av.shape[0], *av.shape[1:]),
                             av.dtype) for av in self.out_avals]
        shard = NamedSharding(self.mesh, PartitionSpec("core"))
        self.dev_in = [self.jax.device_put(a, shard) for a in concat]
        if self.fn is None:
            self._compile_fast()
        return self

    def run(self):
        return self.jax.block_until_ready(self.fn(*self.dev_in))

    def results(self, outs):
        import numpy as _np
        res = []
        for c in range(self.n_cores):
            d = {}
            for i, name in enumerate(self.out_names):
                full = _np.asarray(outs[i])
                d[name] = full.reshape(self.n_cores, *self.out_avals[i].shape)[c]
            res.append(d)
        return res


def make_runner(nc, n_cores):
    return _Runner(nc, n_cores)


def _prep_core_inputs(inputs, b, t, shared):
    x = inputs["x"]
    import ml_dtypes
    bf = ml_dtypes.bfloat16

    if ("xT", b) not in shared:
        shared[("xT", b)] = np.ascontiguousarray(np.asarray(x[b]).T).astype(bf)
    if ("w", t) not in shared:
        Wq, Wk, Wv, Wg, Wo = (np.asarray(inputs[k])
                              for k in ("Wq", "Wk", "Wv", "Wg", "Wo"))
        heads = [8 * t + p for p in _PERM]
        qcols = np.concatenate([np.arange(h * 64, (h + 1) * 64) for h in heads])
        groups = [2 * t, 2 * t + 1]
        kcols = np.concatenate([np.arange(g * 64, (g + 1) * 64) for g in groups])
        shared[("w", t)] = {
            "wq": np.ascontiguousarray(Wq[:, qcols]).astype(bf),
            "wkv": np.ascontiguousarray(
                np.concatenate([Wk[:, kcols], Wv[:, kcols]], axis=1)).astype(bf),
            "wg": np.ascontiguousarray(Wg[:, qcols]).astype(bf),
            "wo": np.ascontiguousarray(Wo[qcols, :]).astype(bf),
        }
    if "const" not in shared:
        q_scale, k_scale = np.asarray(inputs["q_scale"]), np.asarray(inputs["k_scale"])
        cos, sin = np.asarray(inputs["cos"]), np.asarray(inputs["sin"])
        scaling = float(HD) ** -0.5
        tri = (np.arange(128)[:, None] <= np.arange(128)[None, :])
        shared["const"] = {
            "cosd": cos.astype(bf), "sind": sin.astype(bf),
            "qsc": np.broadcast_to(
                np.tile((1.0 + q_scale) * scaling, 8)[None, :],
                (128, 512)).astype(np.float32).copy(),
            "ksc": np.broadcast_to(
                np.tile(1.0 + k_scale, 2)[None, :],
                (128, 128)).astype(np.float32).copy(),
            "trit": tri.astype(bf),
            "ident": np.eye(128, dtype=np.float32).astype(bf),
        }
    return {"xT": shared[("xT", b)], **shared[("w", t)], **shared["const"]}


def kernel(**inputs):
    mask = np.asarray(inputs["mask"])
    classes, tiles = classify_mask(mask)
    key = mask.tobytes()
    if key not in _CACHE:
        nc = build_program(classes, len(tiles))
        _CACHE[key] = (nc, make_runner(nc, NCORES))
    nc, runner = _CACHE[key]

    import ml_dtypes
    mask_arr = (np.stack(tiles).astype(ml_dtypes.bfloat16) if tiles else None)
    shared = {}
    in_maps = []
    for c in range(NCORES):
        m = _prep_core_inputs(inputs, c // 4, c % 4, shared)
        if mask_arr is not None:
            m["maskt"] = mask_arr
        in_maps.append(m)

    runner.prepare(in_maps)
    outs = runner.run()
    res = runner.results(outs)
    out = np.zeros((B, S, D), np.float32)
    for c in range(NCORES):
        out[c // 4] += res[c]["y"].astype(np.float32)
    return out.astype(np.asarray(inputs["x"]).dtype)
